# revision 16
# baseline (speedup 1.0000x reference)
"""GQA attention (B=2, S=2048, DIM=2048, H=16, KVH=4, HD=128, RoPE, causal)
on 8 TRN2 NeuronCores.

Sharding: core c -> batch b = c//4, head-group g = c%4 (q heads 4g..4g+3,
which map exactly to kv head g). Each core computes the partial output
attn_heads @ wo_slice.T  ([S, DIM]); the host sums the 4 partials per batch.

Device layout (everything "transposed", feature-major):
  xT   [DIM, S]   bf16   x[b].T
  wqT  [DIM, 512] bf16   (per-head even/odd-permuted, 1/sqrt(HD)-scaled) wq.T
  wkT  [DIM, 128] bf16   permuted wk.T
  wvT  [DIM, 128] bf16   wv.T (not permuted; v is not roped)
  woT  [512, DIM] bf16   wo[:, cols].T
  cosT [128, S]   bf16   [cos; cos] rope table, frequency-major, duplicated
  sinT [128, S]   bf16   [-sin; sin] sign-folded rope table

The per-head even/odd permutation (rows [0,2,..,126,1,3,..,127]) turns RoPE
pair-interleaving into contiguous half-partitions; q.k dot products are
invariant because q and k are permuted identically.

Attention is computed in transposed score layout: scoresT[k, q] so that
probsT feeds the PV matmul directly (lhsT = v natural layout), attnT falls
out in [hd, q] = exactly the lhsT the output projection needs.

Softmax denominators: full-width prob tiles are pairwise tree-summed on the
DVE (bf16 4x mode), then a single ones-stationary matmul per (head, chunk)
reduces across the 128 key partitions; diagonal partial-width tiles are
accumulated straight into the same PSUM group. This keeps the PE cost of
the denominator at ~1 matmul per chunk instead of 1 per (k-tile, chunk).

x is streamed seq-chunk-major (16 [128,512] tiles per chunk) and the
projections consume it in the same order (K -> V -> Q per chunk) so the PE
starts ~1us in instead of waiting for the whole 8MB of x.
"""

import math
import sys

import numpy as np

try:
    import concourse.bacc as bacc  # noqa: F401
except ImportError:
    sys.path.insert(0, "/opt/trn_rl_repo")

import ml_dtypes
import concourse.bacc as bacc
import concourse.tile as tile
from concourse import mybir
from concourse.bass_utils import run_bass_kernel_spmd
from concourse.bass import _add_dep_helper

BF16 = mybir.dt.bfloat16
F32 = mybir.dt.float32

B, S, DIM = 2, 2048, 2048
H, KVH, HD = 16, 4, 128
N_CORES = 8
P = 128
D_T = DIM // P      # 16 contraction tiles
NH = H // KVH       # 4 q-heads per core
QC = 512            # q-chunk (matmul moving free dim)
QB = S // QC        # 4 q-chunks
S_T = S // P        # 16 s-tiles / k-tiles

_cached = {}


def _build_nc():
    nc = bacc.Bacc("TRN2", target_bir_lowering=False, debug=False,
                   num_devices=N_CORES)
    xT = nc.dram_tensor("xT", [DIM, S], BF16, kind="ExternalInput").ap()
    wqT = nc.dram_tensor("wqT", [DIM, NH * HD], BF16, kind="ExternalInput").ap()
    wkT = nc.dram_tensor("wkT", [DIM, HD], BF16, kind="ExternalInput").ap()
    wvT = nc.dram_tensor("wvT", [DIM, HD], BF16, kind="ExternalInput").ap()
    woT = nc.dram_tensor("woT", [NH * HD, DIM], BF16, kind="ExternalInput").ap()
    cosT = nc.dram_tensor("cosT", [HD, S], BF16, kind="ExternalInput").ap()
    sinT = nc.dram_tensor("sinT", [HD, S], BF16, kind="ExternalInput").ap()
    out = nc.dram_tensor("out", [S, DIM], BF16, kind="ExternalOutput").ap()

    with tile.TileContext(nc) as tc:
        _build_kernel(tc, xT, wqT, wkT, wvT, woT, cosT, sinT, out)
    nc.compile()
    return nc


def _build_kernel(tc, xT, wqT, wkT, wvT, woT, cosT, sinT, out):
    nc = tc.nc
    Exp = mybir.ActivationFunctionType.Exp

    with (
        tc.tile_pool(name="const", bufs=1) as const,
        tc.tile_pool(name="big", bufs=1) as big,
        tc.tile_pool(name="rtmp", bufs=11) as rtmp,
        tc.tile_pool(name="probs", bufs=9) as probs_pool,
        tc.tile_pool(name="zacc", bufs=7) as zacc_pool,
        tc.tile_pool(name="attn", bufs=8) as attn_pool,
        tc.tile_pool(name="rz", bufs=3) as rz_pool,
        tc.tile_pool(name="osb", bufs=6) as osb_pool,
        tc.tile_pool(name="ps", bufs=5, space="PSUM") as ps_pool,
        tc.tile_pool(name="ps_at", bufs=2, space="PSUM") as ps_at_pool,
        tc.tile_pool(name="ps_z", bufs=1, space="PSUM") as ps_z_pool,
    ):
        # ---- constants ----
        ones = const.tile([P, P], BF16, name="ones")
        nc.vector.memset(ones, 1.0)
        # cos_sb = [cos; cos], sin_sb = [-sin; sin] (host-prepared), so the
        # whole rotation is 3 full-width ops on partition-aligned tiles.
        cos_sb = const.tile([HD, S], BF16, name="cos")
        sin_sb = const.tile([HD, S], BF16, name="sin")

        # ---- weights / activations ----
        # DMA descriptors of all pending transfers round-robin across the 16
        # queues, so without dependencies everything arrives "together" near
        # the end. Explicit dep chains (2-wide streams) sequence arrivals:
        # wk + x chunk 0 first (K proj gates the start), then wv, rope
        # tables, wq, remaining x chunks, wo (needed last).
        wk_sb = big.tile([P, D_T, HD], BF16, name="wk")
        wk_r = wkT.rearrange("(t p) j -> p t j", p=P)
        d_wk0 = nc.sync.dma_start(out=wk_sb[:, 0:2, :], in_=wk_r[:, 0:2, :])
        d_wk1 = nc.sync.dma_start(out=wk_sb[:, 2:D_T, :], in_=wk_r[:, 2:D_T, :])
        _add_dep_helper(d_wk1.ins, d_wk0.ins, sync=True, reason="wk order")

        xt_t = [big.tile([P, S], BF16, name=f"xt{dt}") for dt in range(D_T)]

        x_dmas = {}

        def load_x_chunk(sc, deps):
            # 2-wide chained streams within a chunk: arrival tracks dt order
            for dt in range(D_T):
                d = nc.sync.dma_start(
                    out=xt_t[dt][:, sc * QC:(sc + 1) * QC],
                    in_=xT[dt * P:(dt + 1) * P, sc * QC:(sc + 1) * QC])
                parent = x_dmas.get((sc, dt - 2)) if dt >= 2 else deps[dt % len(deps)]
                if parent is not None:
                    _add_dep_helper(d.ins, parent.ins, sync=True,
                                    reason="x stream order")
                x_dmas[(sc, dt)] = d

        load_x_chunk(0, [None, None])

        wv_sb = big.tile([P, D_T, HD], BF16, name="wv")
        d_wv = nc.sync.dma_start(out=wv_sb,
                                 in_=wvT.rearrange("(t p) j -> p t j", p=P))
        _add_dep_helper(d_wv.ins, x_dmas[(0, 7)].ins, sync=True, reason="wv")

        rope_dmas = []
        for _i, (_src, _dst) in enumerate(((cosT, cos_sb), (sinT, sin_sb))):
            _d = nc.sync.dma_start(out=_dst, in_=_src)
            _add_dep_helper(_d.ins, x_dmas[(0, 9 + 2 * _i)].ins, sync=True,
                            reason="rope tables")
            rope_dmas.append(_d)

        wq_sb = big.tile([P, D_T, NH * HD], BF16, name="wq")
        wq_r = wqT.rearrange("(t p) j -> p t j", p=P)
        wq_dmas = []
        for hh in range(NH):
            d = nc.sync.dma_start(
                out=wq_sb[:, :, hh * HD:(hh + 1) * HD],
                in_=wq_r[:, :, hh * HD:(hh + 1) * HD])
            _add_dep_helper(d.ins, x_dmas[(0, 11 + hh)].ins, sync=True,
                            reason="wq after x chunk 0")
            wq_dmas.append(d)

        load_x_chunk(1, wq_dmas[2:])
        load_x_chunk(2, [x_dmas[(1, 14)], x_dmas[(1, 15)]])
        load_x_chunk(3, [x_dmas[(2, 14)], x_dmas[(2, 15)]])

        wo_sb = big.tile([P, NH, DIM], BF16, name="wo")
        d_wo = nc.sync.dma_start(out=wo_sb,
                                 in_=woT.rearrange("(t p) d -> p t d", p=P))
        _add_dep_helper(d_wo.ins, x_dmas[(2, 15)].ins, sync=True, reason="wo")

        xt_tiles = {}
        for dt in range(D_T):
            for sc in range(QB):
                xt_tiles[(dt, sc)] = xt_t[dt][:, sc * QC:(sc + 1) * QC]

        qT = big.tile([P, NH, S], BF16, name="qT")
        kT = big.tile([P, S], BF16, name="kT")
        v_sb = big.tile([P, S_T, HD], BF16, name="v")

        def rope(dst, ps, sc):
            """dst (bf16 [128,512] slice) <- rotate(ps).

            ACT stages ps to bf16 SBUF twice (straight + halves swapped via
            ScalarE partition-shifting copies); DVE then runs three
            full-width 16-bit 2x-mode ops against the sign-folded tables:
            dst = st*[cos;cos] + sw*[-sin;sin]."""
            h = HD // 2
            st = rtmp.tile([P, QC], BF16, name="rst")
            sw = rtmp.tile([P, QC], BF16, name="rsw")
            nc.scalar.copy(out=st, in_=ps)
            nc.scalar.copy(out=sw[0:h, :], in_=ps[h:P, :])
            nc.scalar.copy(out=sw[h:P, :], in_=ps[0:h, :])
            cos_c = cos_sb[:, sc * QC:(sc + 1) * QC]
            sin_c = sin_sb[:, sc * QC:(sc + 1) * QC]
            t0 = rtmp.tile([P, QC], BF16, name="rt")
            t1 = rtmp.tile([P, QC], BF16, name="rt")
            nc.vector.tensor_mul(t0, st, cos_c)
            nc.vector.tensor_mul(t1, sw, sin_c)
            nc.vector.tensor_add(dst, t0, t1)

        # ---- projections, x-chunk-major (follows DMA arrival order) ----
        for sc in range(QB):
            # K projection + rope
            ps = ps_pool.tile([P, QC], F32, name="ps")
            for dt in range(D_T):
                nc.tensor.matmul(ps, lhsT=wk_sb[:, dt, :],
                                 rhs=xt_tiles[(dt, sc)],
                                 start=(dt == 0), stop=(dt == D_T - 1))
            rope(kT[:, sc * QC:(sc + 1) * QC], ps, sc)

            # V projection (natural [s, hd] layout)
            for st in range(4 * sc, 4 * sc + 4):
                ps = ps_pool.tile([P, QC], F32, name="ps")
                for dt in range(D_T):
                    nc.tensor.matmul(
                        ps[:, 0:HD],
                        lhsT=xt_tiles[(dt, sc)][:, (st % 4) * P:(st % 4 + 1) * P],
                        rhs=wv_sb[:, dt, :],
                        start=(dt == 0), stop=(dt == D_T - 1))
                nc.scalar.copy(out=v_sb[:, st, :], in_=ps[:, 0:HD])

            # Q projection + rope
            for hh in range(NH):
                ps = ps_pool.tile([P, QC], F32, name="ps")
                for dt in range(D_T):
                    nc.tensor.matmul(ps, lhsT=wq_sb[:, dt, hh * HD:(hh + 1) * HD],
                                     rhs=xt_tiles[(dt, sc)],
                                     start=(dt == 0), stop=(dt == D_T - 1))
                rope(qT[:, hh, sc * QC:(sc + 1) * QC], ps, sc)

        # ---- attention + output projection, per q-chunk ----
        # The attention inner loop is ACT(exp)-paced: per full k-tile the PE
        # does ~426ns (scores+PV) while ACT needs ~527ns. O-proj matmuls of
        # the PREVIOUS chunk are interleaved into the current chunk's k-steps
        # so the PE's surplus work soaks up the ACT pacing slack instead of
        # running serially with an idle ACT afterwards.
        chunks = [(1536, 512), (1024, 512), (512, 512), (0, 512)]
        pending_oproj = []  # O-proj tile emitters from the previous chunk

        def make_oproj(attn_tiles, q0, st, dc, use_act):
            def emit():
                op_ps = ps_pool.tile([P, QC], F32, name="ps")
                for j in range(NH):
                    nc.tensor.matmul(
                        op_ps, lhsT=attn_tiles[j][:, st * P:(st + 1) * P],
                        rhs=wo_sb[:, j, dc * QC:(dc + 1) * QC],
                        start=(j == 0), stop=(j == NH - 1))
                o_sb = osb_pool.tile([P, QC], BF16, name="osb")
                if use_act:
                    nc.scalar.copy(out=o_sb, in_=op_ps)
                else:
                    nc.vector.tensor_copy(out=o_sb, in_=op_ps)
                nc.sync.dma_start(
                    out=out[q0 + st * P:q0 + st * P + P,
                            dc * QC:(dc + 1) * QC], in_=o_sb)
            return emit

        for ci, (q0, qw) in enumerate(chunks):
            nk = (q0 + qw) // P  # causal k-tiles for this q-chunk
            attn_tiles = {}
            interval = max(1, (NH * nk) // 16)
            kstep = 0

            for hh in range(NH):
                at_ps = ps_at_pool.tile([P, qw], F32, name="at")
                full_prs = []   # (tile, off) with off == 0; tree-summed below
                part_prs = []   # diagonal partial-width tiles (off > 0)
                tree = []       # pending tree level (full-width bf16 tiles)
                for k in range(nk):
                    # On diagonal tiles only columns q0+off.. are causally
                    # valid; every stage is right-aligned to [off:qw].
                    off = max(0, k * P - q0)
                    diag = k * P >= q0
                    w = qw - off
                    sc_ps = ps_pool.tile([P, QC], F32, name="ps")
                    nc.tensor.matmul(sc_ps[:, off:qw],
                                     lhsT=kT[:, k * P:(k + 1) * P],
                                     rhs=qT[:, hh, q0 + off:q0 + qw],
                                     start=True, stop=True)
                    pr = probs_pool.tile([P, QC], BF16, name="pr")
                    nc.scalar.activation(out=pr[:, off:qw], in_=sc_ps[:, off:qw],
                                         func=Exp)
                    if diag:  # zero where c' < r
                        nc.gpsimd.affine_select(
                            out=pr[:, off:qw], in_=pr[:, off:qw],
                            compare_op=mybir.AluOpType.is_ge,
                            fill=0.0, base=0, pattern=[[1, w]],
                            channel_multiplier=-1)
                    nc.tensor.matmul(at_ps[:, off:qw], lhsT=v_sb[:, k, :],
                                     rhs=pr[:, off:qw],
                                     start=(k == 0), stop=(k == nk - 1))
                    kstep += 1
                    if pending_oproj and kstep % interval == 0:
                        pending_oproj.pop(0)()
                    if off == 0:
                        full_prs.append((pr, 0))
                        # eager balanced tree adds on DVE (bf16 4x mode):
                        # binary-counter merging keeps depth ~log2(F) and
                        # amortizes ~1 add per k-step
                        tree.append((pr, 0))
                        while len(tree) >= 2 and tree[-1][1] == tree[-2][1]:
                            s = zacc_pool.tile([P, QC], BF16, name="zs")
                            # alternate rank-0 merges between GpSimd and DVE
                            # to split the tree cost across idle engines
                            if tree[-1][1] == 0 and (k // 2) % 2 == 0:
                                nc.gpsimd.tensor_add(s, tree[-2][0], tree[-1][0])
                            else:
                                nc.vector.tensor_add(s, tree[-2][0], tree[-1][0])
                            tree = tree[:-2] + [(s, tree[-1][1] + 1)]
                    else:
                        part_prs.append((pr, off))
                # finish the tree (merge leftover ranks, lowest first)
                while len(tree) > 1:
                    s = zacc_pool.tile([P, QC], BF16, name="zs")
                    nc.vector.tensor_add(s, tree[-2][0], tree[-1][0])
                    tree = tree[:-2] + [(s, tree[-2][1] + 1)]
                if len(full_prs) > 1:
                    full_prs = [(tree[0][0], 0)]

                # ---- softmax denominator + normalize ----
                # The tree over full tiles finished ~3 k-steps ago (the last
                # 3 k-tiles are diagonal partials), so the PE never waits.
                z_ps = ps_z_pool.tile([P, qw], F32, name="z")
                acc = full_prs[0][0]
                nc.tensor.matmul(z_ps, lhsT=ones, rhs=acc,
                                 start=True, stop=(len(part_prs) == 0))
                for i, (pr, off) in enumerate(part_prs):
                    nc.tensor.matmul(z_ps[:, off:qw], lhsT=ones,
                                     rhs=pr[:, off:qw], start=False,
                                     stop=(i == len(part_prs) - 1))
                rz = rz_pool.tile([P, qw], F32, name="rz")
                nc.vector.reciprocal_approx_fast(out=rz, in_=z_ps)
                a_sb = attn_pool.tile([P, qw], BF16, name="attn")
                nc.vector.tensor_mul(a_sb, at_ps, rz)
                attn_tiles[hh] = a_sb

            # drain any leftover O-proj tiles of the previous chunk, then
            # queue up this chunk's tiles
            while pending_oproj:
                pending_oproj.pop(0)()
            pending_oproj = [
                make_oproj(attn_tiles, q0, st, dc,
                           use_act=(st * 4 + dc) % 4 == 1)
                for st in range(qw // P) for dc in range(DIM // QC)]

        # final chunk's O-proj runs as the tail (PE-only)
        while pending_oproj:
            pending_oproj.pop(0)()


def _get_nc():
    if "nc" not in _cached:
        _cached["nc"] = _build_nc()
    return _cached["nc"]


def _prep_in_maps(x, freqs_cis, wq, wk, wv, wo):
    bf = ml_dtypes.bfloat16
    perm = np.concatenate([np.arange(0, HD, 2), np.arange(1, HD, 2)])
    scale = 1.0 / math.sqrt(HD)
    wq_p = (wq.reshape(H, HD, DIM)[:, perm, :] * scale).astype(np.float32)
    wk_p = wk.reshape(KVH, HD, DIM)[:, perm, :]
    cos_h = np.ascontiguousarray(freqs_cis[:, :, 0].T)  # [64, S]
    sin_h = np.ascontiguousarray(freqs_cis[:, :, 1].T)
    cosT = np.concatenate([cos_h, cos_h], axis=0).astype(bf)   # [128, S]
    sinT = np.concatenate([-sin_h, sin_h], axis=0).astype(bf)

    in_maps = []
    for c in range(N_CORES):
        b, g = c // KVH, c % KVH
        hq = slice(NH * g, NH * (g + 1))
        in_maps.append({
            "xT": np.ascontiguousarray(x[b].T).astype(bf),
            "wqT": np.ascontiguousarray(
                wq_p[hq].reshape(NH * HD, DIM).T).astype(bf),
            "wkT": np.ascontiguousarray(wk_p[g].T).astype(bf),
            "wvT": np.ascontiguousarray(wv[g * HD:(g + 1) * HD].T).astype(bf),
            "woT": np.ascontiguousarray(
                wo[:, NH * HD * g:NH * HD * (g + 1)].T).astype(bf),
            "cosT": cosT,
            "sinT": sinT,
        })
    return in_maps


def _reduce_outputs(results):
    out = np.zeros((B, S, DIM), np.float32)
    for c in range(N_CORES):
        out[c // KVH] += results[c]["out"].astype(np.float32)
    return out


def kernel(x, freqs_cis, wq, wk, wv, wo, _trace=False, _trace_kwargs=None):
    nc = _get_nc()
    x, freqs_cis, wq, wk, wv, wo = (
        np.asarray(a, np.float32) for a in (x, freqs_cis, wq, wk, wv, wo))
    in_maps = _prep_in_maps(x, freqs_cis, wq, wk, wv, wo)
    res = run_bass_kernel_spmd(nc, in_maps, core_ids=list(range(N_CORES)),
                               trace=_trace, **(_trace_kwargs or {}))
    out = _reduce_outputs(res.results)
    if _trace:
        _cached["last_exec_time_ns"] = res.exec_time_ns
        _cached["last_results"] = res
    return out


# revision 18
# speedup vs baseline: 1.3108x; 1.3108x over previous
"""GQA attention (B=2, S=2048, DIM=2048, H=16, KVH=4, HD=128, RoPE, causal)
on 8 TRN2 NeuronCores.

Sharding: core c -> batch b = c//4, head-group g = c%4 (q heads 4g..4g+3,
which map exactly to kv head g). Each core computes the partial output
attn_heads @ wo_slice.T  ([S, DIM]); the host sums the 4 partials per batch.

Device layout (everything "transposed", feature-major):
  xT   [DIM, S]   bf16   x[b].T
  wqT  [DIM, 512] bf16   (per-head even/odd-permuted, 1/sqrt(HD)-scaled) wq.T
  wkT  [DIM, 128] bf16   permuted wk.T
  wvT  [DIM, 128] bf16   wv.T (not permuted; v is not roped)
  woT  [512, DIM] bf16   wo[:, cols].T
  cosT [128, S]   bf16   [cos; cos] rope table, frequency-major, duplicated
  sinT [128, S]   bf16   [-sin; sin] sign-folded rope table

The per-head even/odd permutation (rows [0,2,..,126,1,3,..,127]) turns RoPE
pair-interleaving into contiguous half-partitions; q.k dot products are
invariant because q and k are permuted identically.

Attention is computed in transposed score layout: scoresT[k, q] so that
probsT feeds the PV matmul directly (lhsT = v natural layout), attnT falls
out in [hd, q] = exactly the lhsT the output projection needs.

Softmax denominators: full-width prob tiles are pairwise tree-summed on the
DVE (bf16 4x mode), then a single ones-stationary matmul per (head, chunk)
reduces across the 128 key partitions; diagonal partial-width tiles are
accumulated straight into the same PSUM group. This keeps the PE cost of
the denominator at ~1 matmul per chunk instead of 1 per (k-tile, chunk).

x is streamed seq-chunk-major (16 [128,512] tiles per chunk) and the
projections consume it in the same order (K -> V -> Q per chunk) so the PE
starts ~1us in instead of waiting for the whole 8MB of x.
"""

import math
import sys

import numpy as np

try:
    import concourse.bacc as bacc  # noqa: F401
except ImportError:
    sys.path.insert(0, "/opt/trn_rl_repo")

import ml_dtypes
import concourse.bacc as bacc
import concourse.tile as tile
from concourse import mybir
from concourse.bass_utils import run_bass_kernel_spmd
from concourse.bass import _add_dep_helper

BF16 = mybir.dt.bfloat16
F32 = mybir.dt.float32

B, S, DIM = 2, 2048, 2048
H, KVH, HD = 16, 4, 128
N_CORES = 8
P = 128
D_T = DIM // P      # 16 contraction tiles
NH = H // KVH       # 4 q-heads per core
QC = 512            # q-chunk (matmul moving free dim)
QB = S // QC        # 4 q-chunks
S_T = S // P        # 16 s-tiles / k-tiles

_cached = {}


def _build_nc():
    nc = bacc.Bacc("TRN2", target_bir_lowering=False, debug=False,
                   num_devices=N_CORES)
    xT = nc.dram_tensor("xT", [DIM, S], BF16, kind="ExternalInput").ap()
    wqT = nc.dram_tensor("wqT", [DIM, NH * HD], BF16, kind="ExternalInput").ap()
    wkT = nc.dram_tensor("wkT", [DIM, HD], BF16, kind="ExternalInput").ap()
    wvT = nc.dram_tensor("wvT", [DIM, HD], BF16, kind="ExternalInput").ap()
    woT = nc.dram_tensor("woT", [NH * HD, DIM], BF16, kind="ExternalInput").ap()
    cosT = nc.dram_tensor("cosT", [HD, S], BF16, kind="ExternalInput").ap()
    sinT = nc.dram_tensor("sinT", [HD, S], BF16, kind="ExternalInput").ap()
    out = nc.dram_tensor("out", [S, DIM], BF16, kind="ExternalOutput").ap()

    with tile.TileContext(nc) as tc:
        _build_kernel(tc, xT, wqT, wkT, wvT, woT, cosT, sinT, out)
    nc.compile()
    return nc


def _build_kernel(tc, xT, wqT, wkT, wvT, woT, cosT, sinT, out):
    nc = tc.nc
    Exp = mybir.ActivationFunctionType.Exp

    with (
        tc.tile_pool(name="const", bufs=1) as const,
        tc.tile_pool(name="big", bufs=1) as big,
        tc.tile_pool(name="rtmp", bufs=11) as rtmp,
        tc.tile_pool(name="probs", bufs=9) as probs_pool,
        tc.tile_pool(name="zacc", bufs=7) as zacc_pool,
        tc.tile_pool(name="attn", bufs=8) as attn_pool,
        tc.tile_pool(name="rz", bufs=3) as rz_pool,
        tc.tile_pool(name="osb", bufs=6) as osb_pool,
        tc.tile_pool(name="ps", bufs=5, space="PSUM") as ps_pool,
        tc.tile_pool(name="ps_at", bufs=2, space="PSUM") as ps_at_pool,
        tc.tile_pool(name="ps_z", bufs=1, space="PSUM") as ps_z_pool,
    ):
        # ---- constants ----
        ones = const.tile([P, P], BF16, name="ones")
        nc.vector.memset(ones, 1.0)
        # cos_sb = [cos; cos], sin_sb = [-sin; sin] (host-prepared), so the
        # whole rotation is 3 full-width ops on partition-aligned tiles.
        cos_sb = const.tile([HD, S], BF16, name="cos")
        sin_sb = const.tile([HD, S], BF16, name="sin")

        # ---- weights / activations ----
        # DMA descriptors of all pending transfers round-robin across the 16
        # queues, so without dependencies everything arrives "together" near
        # the end. Explicit dep chains (2-wide streams) sequence arrivals:
        # wk + x chunk 0 first (K proj gates the start), then wv, rope
        # tables, wq, remaining x chunks, wo (needed last).
        wk_sb = big.tile([P, D_T, HD], BF16, name="wk")
        nc.sync.dma_start(out=wk_sb, in_=wkT.rearrange("(t p) j -> p t j", p=P))

        # x lives in one [p, dt, s] tile so a whole seq-chunk (all 16
        # contraction blocks) moves as ONE 2MB DMA. Chunk-level serial
        # chaining sequences arrivals at full bandwidth (transfer time is
        # ~5.6us per hop, far above the per-hop semaphore latency).
        xt_all = big.tile([P, D_T, S], BF16, name="xt")
        x_r = xT.rearrange("(t p) s -> p t s", p=P)

        def xdma(dts, sc, parent):
            d = nc.sync.dma_start(
                out=xt_all[:, dts[0]:dts[1], sc * QC:(sc + 1) * QC],
                in_=x_r[:, dts[0]:dts[1], sc * QC:(sc + 1) * QC])
            if parent is not None:
                _add_dep_helper(d.ins, parent.ins, sync=True, reason="x order")
            return d

        # chunk 0 in three pieces so K-proj starts ~3us in
        d_x0a = xdma((0, 4), 0, None)
        d_x0b = xdma((4, 8), 0, d_x0a)
        d_x0c = xdma((8, 16), 0, d_x0b)

        wv_sb = big.tile([P, D_T, HD], BF16, name="wv")
        d_wv = nc.sync.dma_start(out=wv_sb,
                                 in_=wvT.rearrange("(t p) j -> p t j", p=P))
        _add_dep_helper(d_wv.ins, d_x0b.ins, sync=True, reason="wv")

        rope_dmas = []
        for _src, _dst in ((cosT, cos_sb), (sinT, sin_sb)):
            _d = nc.sync.dma_start(out=_dst, in_=_src)
            _add_dep_helper(_d.ins, d_x0c.ins, sync=True, reason="rope tables")
            rope_dmas.append(_d)

        wq_sb = big.tile([P, D_T, NH * HD], BF16, name="wq")
        wq_r = wqT.rearrange("(t p) j -> p t j", p=P)
        wq_dmas = []
        for hh in range(NH):
            d = nc.sync.dma_start(
                out=wq_sb[:, :, hh * HD:(hh + 1) * HD],
                in_=wq_r[:, :, hh * HD:(hh + 1) * HD])
            _add_dep_helper(d.ins, rope_dmas[hh % 2].ins, sync=True,
                            reason="wq after rope tables")
            wq_dmas.append(d)

        d_x1 = xdma((0, 16), 1, wq_dmas[1])
        d_x2 = xdma((0, 16), 2, d_x1)
        d_x3 = xdma((0, 16), 3, d_x2)

        wo_sb = big.tile([P, NH, DIM], BF16, name="wo")
        d_wo = nc.sync.dma_start(out=wo_sb,
                                 in_=woT.rearrange("(t p) d -> p t d", p=P))
        _add_dep_helper(d_wo.ins, d_x3.ins, sync=True, reason="wo last")

        xt_tiles = {}
        for dt in range(D_T):
            for sc in range(QB):
                xt_tiles[(dt, sc)] = xt_all[:, dt, sc * QC:(sc + 1) * QC]

        qT = big.tile([P, NH, S], BF16, name="qT")
        kT = big.tile([P, S], BF16, name="kT")
        v_sb = big.tile([P, S_T, HD], BF16, name="v")

        def rope(dst, ps, sc):
            """dst (bf16 [128,512] slice) <- rotate(ps).

            ACT stages ps to bf16 SBUF twice (straight + halves swapped via
            ScalarE partition-shifting copies); DVE then runs three
            full-width 16-bit 2x-mode ops against the sign-folded tables:
            dst = st*[cos;cos] + sw*[-sin;sin]."""
            h = HD // 2
            st = rtmp.tile([P, QC], BF16, name="rst")
            sw = rtmp.tile([P, QC], BF16, name="rsw")
            nc.scalar.copy(out=st, in_=ps)
            nc.scalar.copy(out=sw[0:h, :], in_=ps[h:P, :])
            nc.scalar.copy(out=sw[h:P, :], in_=ps[0:h, :])
            cos_c = cos_sb[:, sc * QC:(sc + 1) * QC]
            sin_c = sin_sb[:, sc * QC:(sc + 1) * QC]
            t0 = rtmp.tile([P, QC], BF16, name="rt")
            t1 = rtmp.tile([P, QC], BF16, name="rt")
            nc.vector.tensor_mul(t0, st, cos_c)
            nc.vector.tensor_mul(t1, sw, sin_c)
            nc.vector.tensor_add(dst, t0, t1)

        # ---- projections, x-chunk-major (follows DMA arrival order) ----
        for sc in range(QB):
            # K projection + rope
            ps = ps_pool.tile([P, QC], F32, name="ps")
            for dt in range(D_T):
                nc.tensor.matmul(ps, lhsT=wk_sb[:, dt, :],
                                 rhs=xt_tiles[(dt, sc)],
                                 start=(dt == 0), stop=(dt == D_T - 1))
            rope(kT[:, sc * QC:(sc + 1) * QC], ps, sc)

            # V projection (natural [s, hd] layout)
            for st in range(4 * sc, 4 * sc + 4):
                ps = ps_pool.tile([P, QC], F32, name="ps")
                for dt in range(D_T):
                    nc.tensor.matmul(
                        ps[:, 0:HD],
                        lhsT=xt_tiles[(dt, sc)][:, (st % 4) * P:(st % 4 + 1) * P],
                        rhs=wv_sb[:, dt, :],
                        start=(dt == 0), stop=(dt == D_T - 1))
                nc.scalar.copy(out=v_sb[:, st, :], in_=ps[:, 0:HD])

            # Q projection + rope
            for hh in range(NH):
                ps = ps_pool.tile([P, QC], F32, name="ps")
                for dt in range(D_T):
                    nc.tensor.matmul(ps, lhsT=wq_sb[:, dt, hh * HD:(hh + 1) * HD],
                                     rhs=xt_tiles[(dt, sc)],
                                     start=(dt == 0), stop=(dt == D_T - 1))
                rope(qT[:, hh, sc * QC:(sc + 1) * QC], ps, sc)

        # ---- attention + output projection, per q-chunk ----
        # The attention inner loop is ACT(exp)-paced: per full k-tile the PE
        # does ~426ns (scores+PV) while ACT needs ~527ns. O-proj matmuls of
        # the PREVIOUS chunk are interleaved into the current chunk's k-steps
        # so the PE's surplus work soaks up the ACT pacing slack instead of
        # running serially with an idle ACT afterwards.
        chunks = [(1536, 512), (1024, 512), (512, 512), (0, 512)]
        pending_oproj = []  # O-proj tile emitters from the previous chunk

        def make_oproj(attn_tiles, q0, st, dc, use_act):
            def emit():
                op_ps = ps_pool.tile([P, QC], F32, name="ps")
                for j in range(NH):
                    nc.tensor.matmul(
                        op_ps, lhsT=attn_tiles[j][:, st * P:(st + 1) * P],
                        rhs=wo_sb[:, j, dc * QC:(dc + 1) * QC],
                        start=(j == 0), stop=(j == NH - 1))
                o_sb = osb_pool.tile([P, QC], BF16, name="osb")
                if use_act:
                    nc.scalar.copy(out=o_sb, in_=op_ps)
                else:
                    nc.vector.tensor_copy(out=o_sb, in_=op_ps)
                nc.sync.dma_start(
                    out=out[q0 + st * P:q0 + st * P + P,
                            dc * QC:(dc + 1) * QC], in_=o_sb)
            return emit

        for ci, (q0, qw) in enumerate(chunks):
            nk = (q0 + qw) // P  # causal k-tiles for this q-chunk
            attn_tiles = {}
            interval = max(1, (NH * nk) // 16)
            kstep = 0

            for hh in range(NH):
                at_ps = ps_at_pool.tile([P, qw], F32, name="at")
                full_prs = []   # (tile, off) with off == 0; tree-summed below
                part_prs = []   # diagonal partial-width tiles (off > 0)
                tree = []       # pending tree level (full-width bf16 tiles)
                for k in range(nk):
                    # On diagonal tiles only columns q0+off.. are causally
                    # valid; every stage is right-aligned to [off:qw].
                    off = max(0, k * P - q0)
                    diag = k * P >= q0
                    w = qw - off
                    sc_ps = ps_pool.tile([P, QC], F32, name="ps")
                    nc.tensor.matmul(sc_ps[:, off:qw],
                                     lhsT=kT[:, k * P:(k + 1) * P],
                                     rhs=qT[:, hh, q0 + off:q0 + qw],
                                     start=True, stop=True)
                    pr = probs_pool.tile([P, QC], BF16, name="pr")
                    nc.scalar.activation(out=pr[:, off:qw], in_=sc_ps[:, off:qw],
                                         func=Exp)
                    if diag:  # zero where c' < r
                        nc.gpsimd.affine_select(
                            out=pr[:, off:qw], in_=pr[:, off:qw],
                            compare_op=mybir.AluOpType.is_ge,
                            fill=0.0, base=0, pattern=[[1, w]],
                            channel_multiplier=-1)
                    nc.tensor.matmul(at_ps[:, off:qw], lhsT=v_sb[:, k, :],
                                     rhs=pr[:, off:qw],
                                     start=(k == 0), stop=(k == nk - 1))
                    kstep += 1
                    if pending_oproj and kstep % interval == 0:
                        pending_oproj.pop(0)()
                    if off == 0:
                        full_prs.append((pr, 0))
                        # eager balanced tree adds on DVE (bf16 4x mode):
                        # binary-counter merging keeps depth ~log2(F) and
                        # amortizes ~1 add per k-step
                        tree.append((pr, 0))
                        while len(tree) >= 2 and tree[-1][1] == tree[-2][1]:
                            s = zacc_pool.tile([P, QC], BF16, name="zs")
                            nc.vector.tensor_add(s, tree[-2][0], tree[-1][0])
                            tree = tree[:-2] + [(s, tree[-1][1] + 1)]
                    else:
                        part_prs.append((pr, off))
                # finish the tree (merge leftover ranks, lowest first)
                while len(tree) > 1:
                    s = zacc_pool.tile([P, QC], BF16, name="zs")
                    nc.vector.tensor_add(s, tree[-2][0], tree[-1][0])
                    tree = tree[:-2] + [(s, tree[-2][1] + 1)]
                if len(full_prs) > 1:
                    full_prs = [(tree[0][0], 0)]

                # ---- softmax denominator + normalize ----
                # The tree over full tiles finished ~3 k-steps ago (the last
                # 3 k-tiles are diagonal partials), so the PE never waits.
                z_ps = ps_z_pool.tile([P, qw], F32, name="z")
                acc = full_prs[0][0]
                nc.tensor.matmul(z_ps, lhsT=ones, rhs=acc,
                                 start=True, stop=(len(part_prs) == 0))
                for i, (pr, off) in enumerate(part_prs):
                    nc.tensor.matmul(z_ps[:, off:qw], lhsT=ones,
                                     rhs=pr[:, off:qw], start=False,
                                     stop=(i == len(part_prs) - 1))
                rz = rz_pool.tile([P, qw], F32, name="rz")
                nc.vector.reciprocal_approx_fast(out=rz, in_=z_ps)
                a_sb = attn_pool.tile([P, qw], BF16, name="attn")
                nc.vector.tensor_mul(a_sb, at_ps, rz)
                attn_tiles[hh] = a_sb

            # drain any leftover O-proj tiles of the previous chunk, then
            # queue up this chunk's tiles
            while pending_oproj:
                pending_oproj.pop(0)()
            pending_oproj = [
                make_oproj(attn_tiles, q0, st, dc,
                           use_act=(st * 4 + dc) % 4 == 1)
                for st in range(qw // P) for dc in range(DIM // QC)]

        # final chunk's O-proj runs as the tail (PE-only)
        while pending_oproj:
            pending_oproj.pop(0)()


def _get_nc():
    if "nc" not in _cached:
        _cached["nc"] = _build_nc()
    return _cached["nc"]


def _prep_in_maps(x, freqs_cis, wq, wk, wv, wo):
    bf = ml_dtypes.bfloat16
    perm = np.concatenate([np.arange(0, HD, 2), np.arange(1, HD, 2)])
    scale = 1.0 / math.sqrt(HD)
    wq_p = (wq.reshape(H, HD, DIM)[:, perm, :] * scale).astype(np.float32)
    wk_p = wk.reshape(KVH, HD, DIM)[:, perm, :]
    cos_h = np.ascontiguousarray(freqs_cis[:, :, 0].T)  # [64, S]
    sin_h = np.ascontiguousarray(freqs_cis[:, :, 1].T)
    cosT = np.concatenate([cos_h, cos_h], axis=0).astype(bf)   # [128, S]
    sinT = np.concatenate([-sin_h, sin_h], axis=0).astype(bf)

    in_maps = []
    for c in range(N_CORES):
        b, g = c // KVH, c % KVH
        hq = slice(NH * g, NH * (g + 1))
        in_maps.append({
            "xT": np.ascontiguousarray(x[b].T).astype(bf),
            "wqT": np.ascontiguousarray(
                wq_p[hq].reshape(NH * HD, DIM).T).astype(bf),
            "wkT": np.ascontiguousarray(wk_p[g].T).astype(bf),
            "wvT": np.ascontiguousarray(wv[g * HD:(g + 1) * HD].T).astype(bf),
            "woT": np.ascontiguousarray(
                wo[:, NH * HD * g:NH * HD * (g + 1)].T).astype(bf),
            "cosT": cosT,
            "sinT": sinT,
        })
    return in_maps


def _reduce_outputs(results):
    out = np.zeros((B, S, DIM), np.float32)
    for c in range(N_CORES):
        out[c // KVH] += results[c]["out"].astype(np.float32)
    return out


def kernel(x, freqs_cis, wq, wk, wv, wo, _trace=False, _trace_kwargs=None):
    nc = _get_nc()
    x, freqs_cis, wq, wk, wv, wo = (
        np.asarray(a, np.float32) for a in (x, freqs_cis, wq, wk, wv, wo))
    in_maps = _prep_in_maps(x, freqs_cis, wq, wk, wv, wo)
    res = run_bass_kernel_spmd(nc, in_maps, core_ids=list(range(N_CORES)),
                               trace=_trace, **(_trace_kwargs or {}))
    out = _reduce_outputs(res.results)
    if _trace:
        _cached["last_exec_time_ns"] = res.exec_time_ns
        _cached["last_results"] = res
    return out
